# revision 69
# baseline (speedup 1.0000x reference)
"""DogeDynamicMaskAttention Trainium2 kernel (transposed-scores redesign).

Sharding: 8 cores = 2 batches x 4 head-groups. Core c: batch b=c//4,
head-group g=c%4 -> heads [4g..4g+4), kv heads {2g, 2g+1}.

Design vs previous baseline:
  - scores computed TRANSPOSED [keys, queries]: the dynamic mask row is a
    per-partition (per-key) bias folded into the exp activation for free;
    the P-matrix transposes + f32r casts of the old layout vanish; the
    attn@v matmul consumes exp output directly (keys on partitions).
  - l (softmax denom) via a ones-column stationary matmul accumulated in
    psum; normalize out tiles with reciprocal + gpsimd partition_broadcast
    + one DVE multiply per (head, query-group).
  - projections in bf16 (x and Wq/Wk/Wv/Wdt host-packed contiguous, so
    DMA is large-descriptor); x resident in SBUF, read once.
  - v natural-layout tiles kept in SBUF (no DRAM bounce).
  - dyn/kth bisection identical to baseline (31-step float-bit bisection),
    overlapped under the q/k/v projections; dynT obtained by tiny PE
    transposes instead of a DRAM round trip.
  - fully-masked (degenerate) rows: l==0 detected on host via l output,
    recomputed faithfully in numpy (expected ~1 row per batch*head).
"""
import sys
import numpy as np
import ml_dtypes

BF16NP = ml_dtypes.bfloat16

sys.path.insert(0, "/root/.axon_site/_ro/trn_rl_repo")

import concourse.bass as bass  # noqa: E402,F401
from concourse import bacc  # noqa: E402
import concourse.tile as tile  # noqa: E402
import concourse.mybir as mybir  # noqa: E402
from concourse.bass_utils import run_bass_kernel_spmd  # noqa: E402
from concourse.alu_op_type import AluOpType  # noqa: E402

F32 = mybir.dt.float32
F32R = mybir.dt.float32r
BF16 = mybir.dt.bfloat16
I32 = mybir.dt.int32
AF = mybir.ActivationFunctionType
AX = mybir.AxisListType.X

B, S, HID = 2, 2048, 2048
H, KV, D = 16, 8, 128
HPC, KVPC = 4, 2
GROUPS = H // KV
NUM_DYN = S // 2
SCALING = D ** -0.5
MIN = float(np.finfo(np.float32).min)
BIG = 1.7e38
P = 128
NT = S // P          # 16
NQ = 4
QW = S // NQ         # 512
NCORES = 8

_cache = {}


def _build_program(blkstate):
    key = ("nc", blkstate)
    if key in _cache:
        return _cache[key]
    nvar = _num_varblocks(blkstate)
    nc = bacc.Bacc("TRN2", target_bir_lowering=False, debug=False,
                   num_devices=NCORES)
    dram = {}
    for name, shape, dt in [
            ("xP", [P, NQ * NT * QW], BF16),
            ("xPf", [P, NQ * NT * QW], F32R),
            ("wqP", [P, HPC * NT * P], BF16),
            ("wkP", [P, KVPC * NT * P], BF16),
            ("wvP", [P, KVPC * NT * P], BF16),
            ("wdtvPr", [P, NT * HPC], F32R),
            ("woP", [P, NT * HPC * P], F32),
            ("cosT", [D, S], F32), ("sinT", [D, S], F32),
            ("acol", [HPC, 1], F32),
            ("eye", [P, P], F32), ("perm", [P, P], F32),
            ("varblkT", [P, max(nvar, 1) * P], F32)]:
        dram[name] = nc.dram_tensor(name, shape, dt, kind="ExternalInput").ap()
    outT_d = nc.dram_tensor("outT", [HID, S], F32, kind="ExternalOutput").ap()
    l_d = nc.dram_tensor("l_out", [HPC, S], F32, kind="ExternalOutput").ap()

    with tile.TileContext(nc) as tc:
        _emit(nc, tc, dram, outT_d, l_d, blkstate)
    nc.compile()
    _cache[key] = nc
    return nc


def _num_varblocks(blkstate):
    n = 0
    for t in range(NT):
        for j in range(NT):
            if blkstate[t][j].startswith("V"):
                n = max(n, int(blkstate[t][j][2:]) + 1)
    return n


def _emit(nc, tc, dram, outT_d, l_d, blkstate):
    from contextlib import ExitStack
    ctx = ExitStack()

    # per-tile computed extent (in key chunks): chunks j < extc[t] participate
    extc = []
    for t in range(NT):
        nz = [j for j in range(NT) if blkstate[t][j] != "M"]
        assert nz and min(nz) == 0, "chunk 0 must be active for every tile"
        extc.append(max(nz) + 1)

    consts = ctx.enter_context(tc.tile_pool(name="consts", bufs=1))

    # dt-critical consts first on the sync ring so the dt pass starts
    # immediately; all bulk loads go on the Activation DGE ring.
    wdtv = consts.tile([P, NT * HPC], F32R, name="c_wdtv")
    nc.sync.dma_start(wdtv[:], dram["wdtvPr"])
    acol_t = consts.tile([HPC, 1], F32, name="c_acol")
    nc.sync.dma_start(acol_t[:], dram["acol"])
    onescol_b = consts.tile([P, 1], BF16, name="onescol")
    nc.vector.memset(onescol_b[:], 1.0)
    kthc = consts.tile([HPC, 1], F32, name="kthc")
    nc.vector.memset(kthc[:], float(NUM_DYN) - 0.5)

    eye_r = consts.tile([P, P], F32R, name="cr_eye")
    perm_r = consts.tile([P, P], F32R, name="cr_perm")
    nvar = _num_varblocks(blkstate)
    varblkT = consts.tile([P, max(nvar, 1) * P], F32, name="c_varblkT")

    csp = ctx.enter_context(tc.tile_pool(name="csp", bufs=1))
    cos_t = csp.tile([D, S], F32, name="cos_t")
    sin_t = csp.tile([D, S], F32, name="sin_t")

    cstg = ctx.enter_context(tc.tile_pool(name="cstg", bufs=2))

    def side_consts():
        # side consts on the Activation ring behind the xs loads: keeps
        # their 2.5MB out of the critical first microseconds where the
        # dt x-stream needs every queue
        nc.scalar.dma_start(cos_t[:], dram["cosT"])
        nc.scalar.dma_start(sin_t[:], dram["sinT"])
        nc.scalar.dma_start(varblkT[:], dram["varblkT"])
        for nm, dst in [("eye", eye_r), ("perm", perm_r)]:
            t = cstg.tile([P, P], F32, name=f"s_{nm}", tag="s")
            nc.scalar.dma_start(t[:], dram[nm])
            nc.scalar.copy(dst[:], t[:])

    act = ctx.enter_context(tc.tile_pool(name="act", bufs=1))
    qkro = [act.tile([P, S], F32R, name=f"qro{h}") for h in range(HPC)]
    kro = [act.tile([P, S], F32R, name=f"kro{i}") for i in range(KVPC)]
    vnat = [act.tile([P, NT * P], BF16, name=f"vnat{i}") for i in range(KVPC)]
    dynT = act.tile([P, NT * HPC], F32, name="dynT")

    with ExitStack() as ctx1:
        xsp = ctx1.enter_context(tc.tile_pool(name="xsp", bufs=1))
        xs = [xsp.tile([P, NT * QW], BF16, name=f"xs{sg}")
              for sg in range(NQ)]
        vop = ctx1.enter_context(tc.tile_pool(name="vop", bufs=1))
        vT_own = [vop.tile([P, S], F32R, name=f"vTown{i}") for i in range(KVPC)]
        dt_sb = vop.tile([HPC, S], F32, name="dt_sb")

        # ---- dt pass (f32-accurate: decides the kth mask set) merged ----
        # with the projections; dt chains interleave with v-projections so
        # the PE stays fed while dt's x stream arrives. The dyn chain +
        # bisection is emitted right after the last dt chain so the scalar
        # and DVE queues reach it early (both are in-order engines).
        dyq = ctx1.enter_context(tc.tile_pool(name="dyq", bufs=1))
        kth_f = dyq.tile([HPC, 1], I32, name="kth_f")
        dynrow = dyq.tile([HPC, S], F32R, name="dynrow")
        dyn_t = dyq.tile([HPC, S], F32, name="dyn_t")
        work = dyq.tile([HPC, S], F32, name="work")
        # work is dead after the dyn chain; reuse its storage for the
        # bisection scratch (bf16 view) and later the penalty tile
        scr = work[:].bitcast(BF16)[:, 0:S]
        pen = work

        def emit_dyn_bisect():
            nc.scalar.activation(work[:], dt_sb[:], AF.Exp)
            nc.scalar.activation(work[:], work[:], AF.Ln, bias=1.0)
            nc.scalar.activation(dyn_t[:], work[:], AF.Exp, scale=acol_t[:])
            lo = dyq.tile([HPC, 1], I32, name="lo")
            hi = dyq.tile([HPC, 1], I32, name="hi")
            mid = dyq.tile([HPC, 1], I32, name="mid")
            dlt = dyq.tile([HPC, 1], I32, name="dlt")
            cges = dyq.tile([HPC, 1], I32, name="cges")
            cltv = dyq.tile([HPC, 1], I32, name="cltv")
            cnt = dyq.tile([HPC, 1], F32, name="cnt")
            nc.vector.memset(lo[:], 0)
            nc.vector.memset(hi[:], 0x7F800000)
            for _ in range(31):
                # mid = (lo + hi) >>> 1 (bit values < 2^31 so the unsigned
                # average is exact under logical shift)
                nc.vector.tensor_tensor(mid[:], hi[:], lo[:],
                                        op=AluOpType.add)
                nc.vector.tensor_scalar(mid[:], mid[:], 1, None,
                                        op0=AluOpType.logical_shift_right)
                nc.vector.tensor_scalar(scr, dyn_t[:],
                                        mid[:, 0:1].bitcast(F32), 0.0,
                                        op0=AluOpType.is_lt,
                                        op1=AluOpType.add,
                                        accum_out=cnt[:])
                nc.vector.tensor_scalar(cges[:], kthc[:], cnt[:, 0:1], None,
                                        op0=AluOpType.is_lt)
                nc.vector.tensor_scalar(cltv[:], kthc[:], cnt[:, 0:1], None,
                                        op0=AluOpType.is_ge)
                nc.vector.copy_predicated(hi[:], cges[:], mid[:])
                nc.vector.copy_predicated(lo[:], cltv[:], mid[:])
            nc.vector.tensor_copy(kth_f[:], lo[:])
            nc.vector.tensor_scalar(pen[:], dyn_t[:],
                                    kth_f[:, 0:1].bitcast(F32), -BIG,
                                    op0=AluOpType.is_lt, op1=AluOpType.mult)
            nc.vector.tensor_tensor(dynrow[:], dyn_t[:], pen[:],
                                    op=AluOpType.add)

        # dps=4: all four dt chains get their own psum bank, so no chain
        # waits on a predecessor's DVE drain copy (bufs=2 serialized the
        # chains ~34us apart and pushed the bisection into the attention
        # phase); pps drops to 4 to stay within the 8 banks.
        with tc.tile_pool(name="dps", bufs=4, space="PSUM") as dps, \
             tc.tile_pool(name="dtx", bufs=2) as dtx, \
             tc.tile_pool(name="wp", bufs=2) as wp, \
             tc.tile_pool(name="pjp", bufs=4) as pjp, \
             tc.tile_pool(name="pps", bufs=4, space="PSUM") as pps:

            def emit_dt(sg):
                dtp = dps.tile([HPC, QW], F32, name="dtp", tag="dtp")
                for cc in range(NT):
                    xf = dtx.tile([P, QW], F32R, name="xf", tag="xf")
                    # all xf chunks on the sync ring, AHEAD of the wfull
                    # loads: the scheduler's DMA model then completes the
                    # dt chains before the projections instead of smearing
                    # them across the whole phase (bisection started ~120us
                    # late otherwise)
                    nc.sync.dma_start(
                        xf[:], dram["xPf"][:, (sg * NT + cc) * QW:
                                           (sg * NT + cc + 1) * QW])
                    nc.tensor.matmul(dtp[:], wdtv[:, cc * HPC:(cc + 1) * HPC],
                                     xf[:],
                                     start=(cc == 0), stop=(cc == NT - 1))
                # DVE copy: keeps the in-order scalar queue free for DMA
                # issues and the dyn chain
                nc.vector.tensor_copy(dt_sb[:, sg * QW:(sg + 1) * QW], dtp[:])
                nc.scalar.dma_start(
                    xs[sg][:], dram["xP"][:, sg * NT * QW:(sg + 1) * NT * QW])

            wsrc = {"v": "wvP", "q": "wqP", "k": "wkP"}

            def emit_proj(kind, oi):
                wfull = wp.tile([P, NT * P], BF16, name="wfull", tag="wf")
                nc.sync.dma_start(
                    wfull[:],
                    dram[wsrc[kind]][:, oi * NT * P:(oi + 1) * NT * P])
                for sg in range(NQ):
                    ps = pps.tile([P, QW], F32, name="ps", tag="ps")
                    for cc in range(NT):
                        nc.tensor.matmul(ps[:], wfull[:, cc * P:(cc + 1) * P],
                                         xs[sg][:, cc * QW:(cc + 1) * QW],
                                         start=(cc == 0), stop=(cc == NT - 1))
                    if kind == "v":
                        # scalar engine: its queue reaches these after the
                        # dyn chain, so they never gate the bisection
                        nc.scalar.copy(
                            vT_own[oi][:, sg * QW:(sg + 1) * QW], ps[:])
                    else:
                        pj = pjp.tile([P, QW], F32R, name="pj", tag="pj")
                        nc.scalar.copy(pj[:], ps[:])
                        rh = pps.tile([P, QW], F32, name="rh", tag="ps")
                        nc.tensor.matmul(rh[:], perm_r[:], pj[:],
                                         start=True, stop=True)
                        # gpsimd cannot read PSUM: stage rh through SBUF on
                        # the scalar engine, then do all RoPE elementwise
                        # work on gpsimd (DVE is busy with the bisection and
                        # its in-order queue would pin pjp tiles for ~70us)
                        rhs = pjp.tile([P, QW], F32, name="rhs", tag="pj")
                        nc.scalar.copy(rhs[:], rh[:])
                        t1 = pjp.tile([P, QW], F32, name="t1", tag="pj")
                        nc.gpsimd.tensor_tensor(
                            t1[:], rhs[:], sin_t[:, sg * QW:(sg + 1) * QW],
                            op=AluOpType.mult)
                        t2 = pjp.tile([P, QW], F32, name="t2", tag="pj")
                        nc.gpsimd.tensor_tensor(
                            t2[:], pj[:], cos_t[:, sg * QW:(sg + 1) * QW],
                            op=AluOpType.mult)
                        dstro = (qkro[oi] if kind == "q" else kro[oi])
                        nc.gpsimd.tensor_tensor(
                            dstro[:, sg * QW:(sg + 1) * QW], t1[:], t2[:],
                            op=AluOpType.add)

            # dt chains first (DMA-paced), then the dyn chain + bisection so
            # its scalar/DVE ops sit ahead of all projection copies in the
            # in-order queues; projections follow and overlap the bisection.
            # (Do NOT wrap this in tc.high_priority(): duplicate priorities
            # desync the psum pool-allocation pass from the schedule and
            # produce garbage results.)
            for sg in range(NQ):
                emit_dt(sg)
            emit_dyn_bisect()
            side_consts()
            for kind, oi in [("v", 0), ("v", 1),
                             ("q", 0), ("q", 1), ("q", 2), ("q", 3),
                             ("k", 0), ("k", 1)]:
                emit_proj(kind, oi)

        # ---------------- natural-layout v tiles (SBUF resident) --------
        with tc.tile_pool(name="vps", bufs=4, space="PSUM") as vps:
            for i in range(KVPC):
                for cc in range(NT):
                    pt = vps.tile([P, P], F32, name="vt", tag="vt")
                    nc.tensor.transpose(pt[:].bitcast(F32R),
                                        vT_own[i][:, cc * P:(cc + 1) * P],
                                        eye_r[:])
                    nc.scalar.copy(vnat[i][:, cc * P:(cc + 1) * P], pt[:])

        # dynT transposes last in the PE queue before attention: they wait
        # on the DVE bisection, so anything emitted after them would stall
        # the in-order PE queue (cost a 122us bubble when emitted early).
        with tc.tile_pool(name="dtp2", bufs=1, space="PSUM") as dtp2:
            dyn_ps = dtp2.tile([P, NT * HPC], F32, name="dyn_ps")
            for cc in range(NT):
                nc.tensor.transpose(
                    dyn_ps[:, cc * HPC:(cc + 1) * HPC].bitcast(F32R),
                    dynrow[:, cc * P:(cc + 1) * P], eye_r[0:HPC, 0:HPC])
            nc.scalar.copy(dynT[:], dyn_ps[:])

    # ---------------- attention (transposed scores) + outproj ----------
    # wo resident: loaded once (not once per query-group), via the
    # Activation DGE ring while the first group's attention runs
    wop = ctx.enter_context(tc.tile_pool(name="wop", bufs=1))
    wos = []
    for ht in range(NT):
        wo = wop.tile([P, HPC * P], F32R, name=f"wo{ht}")
        nc.gpsimd.dma_start(
            wo[:], dram["woP"][:, ht * HPC * P:(ht + 1) * HPC * P])
        wos.append(wo)
    with tc.tile_pool(name="scp", bufs=3, space="PSUM") as scp, \
         tc.tile_pool(name="ovl", bufs=2, space="PSUM") as ovl, \
         tc.tile_pool(name="lpp", bufs=1, space="PSUM") as lpp, \
         tc.tile_pool(name="ptp", bufs=3) as ptp, \
         tc.tile_pool(name="atn", bufs=8) as atn, \
         tc.tile_pool(name="lnb", bufs=2) as lnb, \
         tc.tile_pool(name="lnv", bufs=2) as lnv, \
         tc.tile_pool(name="oub", bufs=4) as oub, \
         tc.tile_pool(name="ops", bufs=2, space="PSUM") as ops:
        for grp in range(NQ):
            base = grp * QW
            tiles = list(range(grp * 4, grp * 4 + 4))
            jmax = max(extc[t] for t in tiles)
            at_grp = {}
            for h in range(HPC):
                kv = h // GROUPS
                ovp = ovl.tile([P, QW], F32, name="ovp", tag="ovp")
                lps = lpp.tile([1, QW], F32, name="lps", tag="lps")

                qlos = []
                for j in range(jmax):
                    acts = [t for t in tiles if j < extc[t]]
                    assert acts == tiles[-len(acts):], \
                        "active tiles must be a suffix of the group"
                    qlos.append(acts[0] * P - base)

                def emit_score(j):
                    qlo = qlos[j]
                    sc = scp.tile([P, QW], F32, name="sc", tag="sc")
                    nc.tensor.matmul(
                        sc[:, qlo:QW], kro[kv][:, j * P:(j + 1) * P],
                        qkro[h][:, base + qlo:base + QW],
                        start=True, stop=True, skip_group_check=True)
                    for t in tiles:
                        if j >= extc[t]:
                            continue
                        st = blkstate[t][j]
                        if st.startswith("V"):
                            vi = int(st[2:])
                            off = t * P - base
                            nc.vector.tensor_tensor(
                                sc[:, off:off + P], sc[:, off:off + P],
                                varblkT[:, vi * P:(vi + 1) * P],
                                op=AluOpType.add)
                    pt = ptp.tile([P, QW], BF16, name="pt", tag="pt")
                    nc.scalar.activation(
                        pt[:, qlo:QW], sc[:, qlo:QW], AF.Exp,
                        bias=dynT[:, j * HPC + h:j * HPC + h + 1])
                    return pt

                # software-pipeline by two chunks: emit chunk j+1/j+2's
                # score matmuls before chunk j's l/av matmuls so the PE
                # works through the exp latency instead of waiting on it.
                ptq = [emit_score(0)]
                if jmax > 1:
                    ptq.append(emit_score(1))
                for j in range(jmax):
                    pt, qlo = ptq.pop(0), qlos[j]
                    if j + 2 < jmax:
                        ptq.append(emit_score(j + 2))
                    nc.tensor.matmul(
                        lps[:, qlo:QW], onescol_b[:], pt[:, qlo:QW],
                        start=(j == 0), stop=(j == jmax - 1),
                        skip_group_check=True)
                    nc.tensor.matmul(
                        ovp[:, qlo:QW], vnat[kv][:, j * P:(j + 1) * P],
                        pt[:, qlo:QW],
                        start=(j == 0), stop=(j == jmax - 1),
                        skip_group_check=True)
                lsb = lnv.tile([1, QW], F32, name="lsb", tag="lv")
                nc.scalar.copy(lsb[:], lps[:])
                nc.sync.dma_start(l_d[h:h + 1, base:base + QW], lsb[:])
                linv = lnv.tile([1, QW], F32, name="linv", tag="lv")
                nc.vector.reciprocal_approx_fast(linv[:], lps[:])
                lbc = lnb.tile([P, QW], F32, name="lbc", tag="lb")
                nc.gpsimd.partition_broadcast(lbc[:], linv[:])
                at = atn.tile([P, QW], F32R, name="at", tag="at")
                nc.vector.tensor_tensor(at[:], ovp[:], lbc[:],
                                        op=AluOpType.mult)
                at_grp[h] = at
            for ht in range(NT):
                op = ops.tile([P, QW], F32, name="op", tag="op")
                for h in range(HPC):
                    nc.tensor.matmul(op[:], wos[ht][:, h * P:(h + 1) * P],
                                     at_grp[h][:], start=(h == 0),
                                     stop=(h == HPC - 1))
                osb = oub.tile([P, QW], F32, name="osb", tag="ob")
                # alternate the psum drain between scalar and DVE so
                # neither in-order queue delays the next group's exps
                if ht % 2 == 0:
                    nc.scalar.copy(osb[:], op[:])
                else:
                    nc.vector.tensor_copy(osb[:], op[:])
                nc.sync.dma_start(
                    outT_d[ht * P:(ht + 1) * P, base:base + QW], osb[:])
    ctx.close()


def _host_prep(hidden_states, cos, sin, attention_mask, Wq, Wk, Wv, A, Wdt, Wo):
    eye = np.eye(P, dtype=np.float32)
    perm = np.zeros((P, P), dtype=np.float32)
    for j in range(64):
        perm[j + 64, j] = -1.0
        perm[j, j + 64] = 1.0

    def pack_w(wT, nblk):
        # wT [HID, nblk*P] f32 -> [P, nblk*NT*P] bf16:
        # [p, (oi*NT+c)*P+f] = wT[c*P+p, oi*P+f]
        w4 = wT.reshape(NT, P, nblk, P)            # [c, p, oi, f]
        return np.ascontiguousarray(
            w4.transpose(1, 2, 0, 3).reshape(P, nblk * NT * P)
        ).astype(np.float32)

    in_maps = []
    blkstates = []
    for c in range(NCORES):
        b, g = divmod(c, 4)
        heads = list(range(4 * g, 4 * g + 4))
        # x packed: [p, ((sg*NT)+c)*QW+f] = x[b][sg*QW+f, c*P+p]
        xb = np.asarray(hidden_states[b], dtype=np.float32)
        xP = np.ascontiguousarray(
            xb.reshape(NQ, QW, NT, P).transpose(3, 0, 2, 1)
            .reshape(P, NQ * NT * QW))
        wqT = (Wq[4 * g * D:(4 * g + 4) * D] * np.float32(SCALING)).T
        wkT = Wk[2 * g * D:(2 * g + 2) * D].T
        wvT = Wv[2 * g * D:(2 * g + 2) * D].T
        wqP = pack_w(np.ascontiguousarray(wqT), HPC)
        wkP = pack_w(np.ascontiguousarray(wkT), KVPC)
        wvP = pack_w(np.ascontiguousarray(wvT), KVPC)
        wdtvT = np.ascontiguousarray(
            (Wdt[heads].astype(np.float64) @ Wv.astype(np.float64))
            .T.astype(np.float32))                 # [HID, 4]
        wdtvP = np.ascontiguousarray(
            wdtvT.reshape(NT, P, HPC).transpose(1, 0, 2).reshape(P, NT * HPC))
        # woP: [p, (ht*HPC+h)*P+f] = WoT[h*P+p, ht*P+f]
        woT = np.ascontiguousarray(Wo[:, 4 * g * D:(4 * g + 4) * D].T)
        woP = np.ascontiguousarray(
            woT.reshape(HPC, P, NT, P).transpose(1, 2, 0, 3)
            .reshape(P, NT * HPC * P))
        acol = A[heads].astype(np.float32).reshape(HPC, 1)
        cosT = np.ascontiguousarray(cos[b].T)
        sinT = np.ascontiguousarray(sin[b].T)
        m = attention_mask[b, 0]
        mb = np.asarray(m).reshape(NT, P, NT, P)
        blkrows = []
        varlist = []
        varkeys = {}
        for t in range(NT):
            row = []
            for j in range(NT):
                blkv = mb[t, :, j, :]
                if np.all(blkv == 0):
                    row.append("Z")
                elif np.all(blkv <= -1e30):
                    row.append("M")
                else:
                    bT = np.ascontiguousarray(
                        np.maximum(blkv, -BIG).T)  # [key, query]
                    kk = bT.tobytes()
                    if kk not in varkeys:
                        varkeys[kk] = len(varlist)
                        varlist.append(bT)
                    row.append(f"V:{varkeys[kk]}")
            # interior M blocks (before a later non-M block) become varying
            nz = [j for j in range(NT) if row[j] != "M"]
            lim = (max(nz) + 1) if nz else 0
            for j in range(lim):
                if row[j] == "M":
                    bT = np.full((P, P), -BIG, np.float32)
                    kk = bT.tobytes()
                    if kk not in varkeys:
                        varkeys[kk] = len(varlist)
                        varlist.append(bT)
                    row[j] = f"V:{varkeys[kk]}"
            blkrows.append(tuple(row))
        if len(varlist) > 8:
            raise NotImplementedError("too many varying mask blocks")
        varblkT = np.zeros((P, max(len(varlist), 1) * P), dtype=np.float32)
        for vi, blkv in enumerate(varlist):
            varblkT[:, vi * P:(vi + 1) * P] = blkv
        blkstate = tuple(blkrows)
        in_maps.append({
            "xP": xP.astype(BF16NP), "xPf": xP,
            "wqP": wqP.astype(BF16NP),
            "wkP": wkP.astype(BF16NP), "wvP": wvP.astype(BF16NP),
            "wdtvPr": wdtvP, "woP": woP, "cosT": cosT,
            "sinT": sinT, "acol": acol, "eye": eye, "perm": perm,
            "varblkT": varblkT,
        })
        blkstates.append(blkstate)
    if len(set(blkstates)) != 1:
        raise NotImplementedError("mask structure differs across batches")
    return in_maps, blkstates[0]


def _softplus64(x):
    x = x.astype(np.float64)
    return np.log1p(np.exp(-np.abs(x))) + np.maximum(x, 0)


def _repair_rows(out, bad, inputs):
    """Recompute rows flagged bad [B, S] with faithful numpy reference math."""
    if not bad.any():
        return out
    hs = inputs["hidden_states"]; cos = inputs["cos"]; sin = inputs["sin"]
    am = inputs["attention_mask"]; Wq = inputs["Wq"]; Wk = inputs["Wk"]
    Wv = inputs["Wv"]; A = inputs["A"]; Wdt = inputs["Wdt"]; Wo = inputs["Wo"]

    def rope(x, c, s):
        x1, x2 = x[..., :D // 2], x[..., D // 2:]
        return x * c + np.concatenate([-x2, x1], axis=-1) * s

    for b in range(B):
        rows = np.where(bad[b])[0]
        if len(rows) == 0:
            continue
        x = hs[b].astype(np.float32)
        k = (x @ Wk.T).reshape(S, KV, D)
        v = (x @ Wv.T).reshape(S, KV, D)
        k = rope(k, cos[b][:, None, :], sin[b][:, None, :])
        v_flat = v.reshape(S, KV * D)
        dt = v_flat @ Wdt.T
        dyn = np.exp(A[None, :] * _softplus64(dt)).astype(np.float32).T
        kth = np.sort(dyn, axis=-1)[:, NUM_DYN - 1:NUM_DYN]
        dmask = np.where(dyn < kth, MIN, dyn).astype(np.float32)
        for s_i in rows:
            q_row = (x[s_i] @ Wq.T).reshape(H, D)
            q_row = rope(q_row, cos[b][s_i][None, :], sin[b][s_i][None, :])
            attn_row = np.zeros((H, D), dtype=np.float32)
            for h in range(H):
                kvh = h // GROUPS
                sc = ((q_row[h] @ k[:, kvh].T) * np.float32(SCALING)
                      + (dmask[h] + am[b, 0, s_i])).astype(np.float32)
                w = np.exp(sc - sc.max())
                w = (w / w.sum()).astype(np.float32)
                attn_row[h] = w @ v[:, kvh]
            out[b, s_i] = attn_row.reshape(H * D) @ Wo.T
    return out


def kernel(**inputs):
    inputs = {k: np.asarray(v) for k, v in inputs.items()}
    in_maps, blkstate = _host_prep(**inputs)
    nc = _build_program(blkstate)
    res = run_bass_kernel_spmd(nc, in_maps, list(range(NCORES)))
    out = np.zeros((B, S, HID), dtype=np.float32)
    bad = np.zeros((B, S), dtype=bool)
    for c in range(NCORES):
        b = c // 4
        out[b] += res.results[c]["outT"].T
        bad[b] |= (res.results[c]["l_out"] == 0).any(axis=0)
    bad |= ~np.isfinite(out).all(axis=2)
    out = _repair_rows(out, bad, inputs)
    return out


# revision 71
# speedup vs baseline: 1.0519x; 1.0519x over previous
"""DogeDynamicMaskAttention Trainium2 kernel (transposed-scores redesign).

Sharding: 8 cores = 2 batches x 4 head-groups. Core c: batch b=c//4,
head-group g=c%4 -> heads [4g..4g+4), kv heads {2g, 2g+1}.

Design vs previous baseline:
  - scores computed TRANSPOSED [keys, queries]: the dynamic mask row is a
    per-partition (per-key) bias folded into the exp activation for free;
    the P-matrix transposes + f32r casts of the old layout vanish; the
    attn@v matmul consumes exp output directly (keys on partitions).
  - l (softmax denom) via a ones-column stationary matmul accumulated in
    psum; normalize out tiles with reciprocal + gpsimd partition_broadcast
    + one DVE multiply per (head, query-group).
  - projections in bf16 (x and Wq/Wk/Wv/Wdt host-packed contiguous, so
    DMA is large-descriptor); x resident in SBUF, read once.
  - v natural-layout tiles kept in SBUF (no DRAM bounce).
  - dyn/kth bisection identical to baseline (31-step float-bit bisection),
    overlapped under the q/k/v projections; dynT obtained by tiny PE
    transposes instead of a DRAM round trip.
  - fully-masked (degenerate) rows: l==0 detected on host via l output,
    recomputed faithfully in numpy (expected ~1 row per batch*head).
"""
import sys
import numpy as np
import ml_dtypes

BF16NP = ml_dtypes.bfloat16

sys.path.insert(0, "/root/.axon_site/_ro/trn_rl_repo")

import concourse.bass as bass  # noqa: E402,F401
from concourse import bacc  # noqa: E402
import concourse.tile as tile  # noqa: E402
import concourse.mybir as mybir  # noqa: E402
from concourse.bass_utils import run_bass_kernel_spmd  # noqa: E402
from concourse.alu_op_type import AluOpType  # noqa: E402

F32 = mybir.dt.float32
F32R = mybir.dt.float32r
BF16 = mybir.dt.bfloat16
I32 = mybir.dt.int32
AF = mybir.ActivationFunctionType
AX = mybir.AxisListType.X

B, S, HID = 2, 2048, 2048
H, KV, D = 16, 8, 128
HPC, KVPC = 4, 2
GROUPS = H // KV
NUM_DYN = S // 2
SCALING = D ** -0.5
MIN = float(np.finfo(np.float32).min)
BIG = 1.7e38
P = 128
NT = S // P          # 16
NQ = 4
QW = S // NQ         # 512
NCORES = 8

_cache = {}


def _build_program(blkstate):
    key = ("nc", blkstate)
    if key in _cache:
        return _cache[key]
    nvar = _num_varblocks(blkstate)
    nc = bacc.Bacc("TRN2", target_bir_lowering=False, debug=False,
                   num_devices=NCORES)
    dram = {}
    for name, shape, dt in [
            ("xP", [P, NQ * NT * QW], BF16),
            ("xPf", [P, NQ * NT * QW], F32R),
            ("wqP", [P, HPC * NT * P], BF16),
            ("wkP", [P, KVPC * NT * P], BF16),
            ("wvP", [P, KVPC * NT * P], BF16),
            ("wdtvPr", [P, NT * HPC], F32R),
            ("woP", [P, NT * HPC * P], F32),
            ("cosT", [D, S], F32), ("sinT", [D, S], F32),
            ("acol", [HPC, 1], F32),
            ("eye", [P, P], F32), ("perm", [P, P], F32),
            ("varblkT", [P, max(nvar, 1) * P], F32)]:
        dram[name] = nc.dram_tensor(name, shape, dt, kind="ExternalInput").ap()
    outT_d = nc.dram_tensor("outT", [HID, S], F32, kind="ExternalOutput").ap()
    l_d = nc.dram_tensor("l_out", [HPC, S], F32, kind="ExternalOutput").ap()

    with tile.TileContext(nc) as tc:
        _emit(nc, tc, dram, outT_d, l_d, blkstate)
    nc.compile()
    _cache[key] = nc
    return nc


def _num_varblocks(blkstate):
    n = 0
    for t in range(NT):
        for j in range(NT):
            if blkstate[t][j].startswith("V"):
                n = max(n, int(blkstate[t][j][2:]) + 1)
    return n


def _emit(nc, tc, dram, outT_d, l_d, blkstate):
    from contextlib import ExitStack
    ctx = ExitStack()

    # per-tile computed extent (in key chunks): chunks j < extc[t] participate
    extc = []
    for t in range(NT):
        nz = [j for j in range(NT) if blkstate[t][j] != "M"]
        assert nz and min(nz) == 0, "chunk 0 must be active for every tile"
        extc.append(max(nz) + 1)

    consts = ctx.enter_context(tc.tile_pool(name="consts", bufs=1))

    # dt-critical consts first on the sync ring so the dt pass starts
    # immediately; all bulk loads go on the Activation DGE ring.
    wdtv = consts.tile([P, NT * HPC], F32R, name="c_wdtv")
    nc.sync.dma_start(wdtv[:], dram["wdtvPr"])
    acol_t = consts.tile([HPC, 1], F32, name="c_acol")
    nc.sync.dma_start(acol_t[:], dram["acol"])
    onescol_b = consts.tile([P, 1], BF16, name="onescol")
    nc.vector.memset(onescol_b[:], 1.0)
    kthc = consts.tile([HPC, 1], F32, name="kthc")
    nc.vector.memset(kthc[:], float(NUM_DYN) - 0.5)

    eye_r = consts.tile([P, P], F32R, name="cr_eye")
    perm_r = consts.tile([P, P], F32R, name="cr_perm")
    nvar = _num_varblocks(blkstate)
    varblkT = consts.tile([P, max(nvar, 1) * P], F32, name="c_varblkT")

    csp = ctx.enter_context(tc.tile_pool(name="csp", bufs=1))
    cos_t = csp.tile([D, S], F32, name="cos_t")
    sin_t = csp.tile([D, S], F32, name="sin_t")

    cstg = ctx.enter_context(tc.tile_pool(name="cstg", bufs=2))

    def side_consts():
        # side consts on the Activation ring behind the xs loads: keeps
        # their 2.5MB out of the critical first microseconds where the
        # dt x-stream needs every queue
        nc.scalar.dma_start(cos_t[:], dram["cosT"])
        nc.scalar.dma_start(sin_t[:], dram["sinT"])
        nc.scalar.dma_start(varblkT[:], dram["varblkT"])
        for nm, dst in [("eye", eye_r), ("perm", perm_r)]:
            t = cstg.tile([P, P], F32, name=f"s_{nm}", tag="s")
            nc.scalar.dma_start(t[:], dram[nm])
            nc.scalar.copy(dst[:], t[:])

    act = ctx.enter_context(tc.tile_pool(name="act", bufs=1))
    qkro = [act.tile([P, S], F32R, name=f"qro{h}") for h in range(HPC)]
    kro = [act.tile([P, S], F32R, name=f"kro{i}") for i in range(KVPC)]
    vnat = [act.tile([P, NT * P], BF16, name=f"vnat{i}") for i in range(KVPC)]
    dynT = act.tile([P, NT * HPC], F32, name="dynT")

    with ExitStack() as ctx1:
        xsp = ctx1.enter_context(tc.tile_pool(name="xsp", bufs=1))
        xs = [xsp.tile([P, NT * QW], BF16, name=f"xs{sg}")
              for sg in range(NQ)]
        vop = ctx1.enter_context(tc.tile_pool(name="vop", bufs=1))
        vT_own = [vop.tile([P, S], F32R, name=f"vTown{i}") for i in range(KVPC)]
        dt_sb = vop.tile([HPC, S], F32, name="dt_sb")

        # ---- dt pass (f32-accurate: decides the kth mask set) merged ----
        # with the projections; dt chains interleave with v-projections so
        # the PE stays fed while dt's x stream arrives. The dyn chain +
        # bisection is emitted right after the last dt chain so the scalar
        # and DVE queues reach it early (both are in-order engines).
        dyq = ctx1.enter_context(tc.tile_pool(name="dyq", bufs=1))
        kth_f = dyq.tile([HPC, 1], I32, name="kth_f")
        dynrow = dyq.tile([HPC, S], F32R, name="dynrow")
        dyn_t = dyq.tile([HPC, S], F32, name="dyn_t")
        work = dyq.tile([HPC, S], F32, name="work")
        # work is dead after the dyn chain; reuse its storage for the
        # bisection scratch (bf16 view) and later the penalty tile
        scr = work[:].bitcast(BF16)[:, 0:S]
        pen = work

        def emit_dyn_bisect():
            nc.scalar.activation(work[:], dt_sb[:], AF.Exp)
            nc.scalar.activation(work[:], work[:], AF.Ln, bias=1.0)
            nc.scalar.activation(dyn_t[:], work[:], AF.Exp, scale=acol_t[:])
            lo = dyq.tile([HPC, 1], I32, name="lo")
            hi = dyq.tile([HPC, 1], I32, name="hi")
            mid = dyq.tile([HPC, 1], I32, name="mid")
            dlt = dyq.tile([HPC, 1], I32, name="dlt")
            cges = dyq.tile([HPC, 1], I32, name="cges")
            cltv = dyq.tile([HPC, 1], I32, name="cltv")
            cnt = dyq.tile([HPC, 1], F32, name="cnt")
            # init to [1e-4, 1e4]: dyn = exp(A*softplus(dt)) stays inside
            # unless |A*softplus(dt)| > 9.2 (measured max 0.94 — 10x log
            # margin); the 2^27.8-bit span converges exactly in 28 steps
            nc.vector.memset(lo[:], 0x38D1B717)
            nc.vector.memset(hi[:], 0x461C4000)
            for _ in range(28):
                # mid = (lo + hi) >>> 1 (bit values < 2^31 so the unsigned
                # average is exact under logical shift)
                nc.vector.tensor_tensor(mid[:], hi[:], lo[:],
                                        op=AluOpType.add)
                nc.vector.tensor_scalar(mid[:], mid[:], 1, None,
                                        op0=AluOpType.logical_shift_right)
                nc.vector.tensor_scalar(scr, dyn_t[:],
                                        mid[:, 0:1].bitcast(F32), 0.0,
                                        op0=AluOpType.is_lt,
                                        op1=AluOpType.add,
                                        accum_out=cnt[:])
                nc.vector.tensor_scalar(cges[:], kthc[:], cnt[:, 0:1], None,
                                        op0=AluOpType.is_lt)
                nc.vector.tensor_scalar(cltv[:], kthc[:], cnt[:, 0:1], None,
                                        op0=AluOpType.is_ge)
                nc.vector.copy_predicated(hi[:], cges[:], mid[:])
                nc.vector.copy_predicated(lo[:], cltv[:], mid[:])
            nc.vector.tensor_copy(kth_f[:], lo[:])
            nc.vector.tensor_scalar(pen[:], dyn_t[:],
                                    kth_f[:, 0:1].bitcast(F32), -BIG,
                                    op0=AluOpType.is_lt, op1=AluOpType.mult)
            nc.vector.tensor_tensor(dynrow[:], dyn_t[:], pen[:],
                                    op=AluOpType.add)

        with tc.tile_pool(name="dps", bufs=2, space="PSUM") as dps, \
             tc.tile_pool(name="dtx", bufs=2) as dtx, \
             tc.tile_pool(name="wp", bufs=2) as wp, \
             tc.tile_pool(name="pjp", bufs=4) as pjp, \
             tc.tile_pool(name="pps", bufs=6, space="PSUM") as pps:

            def emit_dt(sg):
                dtp = dps.tile([HPC, QW], F32, name="dtp", tag="dtp")
                for cc in range(NT):
                    xf = dtx.tile([P, QW], F32R, name="xf", tag="xf")
                    # all xf chunks on the sync ring, AHEAD of the wfull
                    # loads: the scheduler's DMA model then completes the
                    # dt chains before the projections instead of smearing
                    # them across the whole phase (bisection started ~120us
                    # late otherwise)
                    nc.sync.dma_start(
                        xf[:], dram["xPf"][:, (sg * NT + cc) * QW:
                                           (sg * NT + cc + 1) * QW])
                    nc.tensor.matmul(dtp[:], wdtv[:, cc * HPC:(cc + 1) * HPC],
                                     xf[:],
                                     start=(cc == 0), stop=(cc == NT - 1))
                # DVE copy: keeps the in-order scalar queue free for DMA
                # issues and the dyn chain
                nc.vector.tensor_copy(dt_sb[:, sg * QW:(sg + 1) * QW], dtp[:])
                nc.scalar.dma_start(
                    xs[sg][:], dram["xP"][:, sg * NT * QW:(sg + 1) * NT * QW])

            wsrc = {"v": "wvP", "q": "wqP", "k": "wkP"}

            def emit_proj(kind, oi):
                wfull = wp.tile([P, NT * P], BF16, name="wfull", tag="wf")
                nc.sync.dma_start(
                    wfull[:],
                    dram[wsrc[kind]][:, oi * NT * P:(oi + 1) * NT * P])
                for sg in range(NQ):
                    ps = pps.tile([P, QW], F32, name="ps", tag="ps")
                    for cc in range(NT):
                        nc.tensor.matmul(ps[:], wfull[:, cc * P:(cc + 1) * P],
                                         xs[sg][:, cc * QW:(cc + 1) * QW],
                                         start=(cc == 0), stop=(cc == NT - 1))
                    if kind == "v":
                        # scalar engine: its queue reaches these after the
                        # dyn chain, so they never gate the bisection
                        nc.scalar.copy(
                            vT_own[oi][:, sg * QW:(sg + 1) * QW], ps[:])
                    else:
                        pj = pjp.tile([P, QW], F32R, name="pj", tag="pj")
                        nc.scalar.copy(pj[:], ps[:])
                        rh = pps.tile([P, QW], F32, name="rh", tag="ps")
                        nc.tensor.matmul(rh[:], perm_r[:], pj[:],
                                         start=True, stop=True)
                        # gpsimd cannot read PSUM: stage rh through SBUF on
                        # the scalar engine, then do all RoPE elementwise
                        # work on gpsimd (DVE is busy with the bisection and
                        # its in-order queue would pin pjp tiles for ~70us)
                        rhs = pjp.tile([P, QW], F32, name="rhs", tag="pj")
                        nc.scalar.copy(rhs[:], rh[:])
                        t1 = pjp.tile([P, QW], F32, name="t1", tag="pj")
                        nc.gpsimd.tensor_tensor(
                            t1[:], rhs[:], sin_t[:, sg * QW:(sg + 1) * QW],
                            op=AluOpType.mult)
                        t2 = pjp.tile([P, QW], F32, name="t2", tag="pj")
                        nc.gpsimd.tensor_tensor(
                            t2[:], pj[:], cos_t[:, sg * QW:(sg + 1) * QW],
                            op=AluOpType.mult)
                        dstro = (qkro[oi] if kind == "q" else kro[oi])
                        nc.gpsimd.tensor_tensor(
                            dstro[:, sg * QW:(sg + 1) * QW], t1[:], t2[:],
                            op=AluOpType.add)

            # dt chains first (DMA-paced), then the dyn chain + bisection so
            # its scalar/DVE ops sit ahead of all projection copies in the
            # in-order queues; projections follow and overlap the bisection.
            # (Do NOT wrap this in tc.high_priority(): duplicate priorities
            # desync the psum pool-allocation pass from the schedule and
            # produce garbage results.)
            for sg in range(NQ):
                emit_dt(sg)
            emit_dyn_bisect()
            side_consts()
            for kind, oi in [("v", 0), ("v", 1),
                             ("q", 0), ("q", 1), ("q", 2), ("q", 3),
                             ("k", 0), ("k", 1)]:
                emit_proj(kind, oi)

        # ---------------- natural-layout v tiles (SBUF resident) --------
        with tc.tile_pool(name="vps", bufs=4, space="PSUM") as vps:
            for i in range(KVPC):
                for cc in range(NT):
                    pt = vps.tile([P, P], F32, name="vt", tag="vt")
                    nc.tensor.transpose(pt[:].bitcast(F32R),
                                        vT_own[i][:, cc * P:(cc + 1) * P],
                                        eye_r[:])
                    nc.scalar.copy(vnat[i][:, cc * P:(cc + 1) * P], pt[:])

        # dynT transposes last in the PE queue before attention: they wait
        # on the DVE bisection, so anything emitted after them would stall
        # the in-order PE queue (cost a 122us bubble when emitted early).
        with tc.tile_pool(name="dtp2", bufs=1, space="PSUM") as dtp2:
            dyn_ps = dtp2.tile([P, NT * HPC], F32, name="dyn_ps")
            for cc in range(NT):
                nc.tensor.transpose(
                    dyn_ps[:, cc * HPC:(cc + 1) * HPC].bitcast(F32R),
                    dynrow[:, cc * P:(cc + 1) * P], eye_r[0:HPC, 0:HPC])
            nc.scalar.copy(dynT[:], dyn_ps[:])

    # ---------------- attention (transposed scores) + outproj ----------
    # wo resident: loaded once (not once per query-group), via the
    # Activation DGE ring while the first group's attention runs
    wop = ctx.enter_context(tc.tile_pool(name="wop", bufs=1))
    wos = []
    for ht in range(NT):
        wo = wop.tile([P, HPC * P], F32R, name=f"wo{ht}")
        nc.gpsimd.dma_start(
            wo[:], dram["woP"][:, ht * HPC * P:(ht + 1) * HPC * P])
        wos.append(wo)
    with tc.tile_pool(name="scp", bufs=3, space="PSUM") as scp, \
         tc.tile_pool(name="ovl", bufs=2, space="PSUM") as ovl, \
         tc.tile_pool(name="lpp", bufs=1, space="PSUM") as lpp, \
         tc.tile_pool(name="ptp", bufs=3) as ptp, \
         tc.tile_pool(name="atn", bufs=8) as atn, \
         tc.tile_pool(name="lnb", bufs=2) as lnb, \
         tc.tile_pool(name="lnv", bufs=2) as lnv, \
         tc.tile_pool(name="oub", bufs=4) as oub, \
         tc.tile_pool(name="ops", bufs=2, space="PSUM") as ops:
        for grp in range(NQ):
            base = grp * QW
            tiles = list(range(grp * 4, grp * 4 + 4))
            jmax = max(extc[t] for t in tiles)
            at_grp = {}
            for h in range(HPC):
                kv = h // GROUPS
                ovp = ovl.tile([P, QW], F32, name="ovp", tag="ovp")
                lps = lpp.tile([1, QW], F32, name="lps", tag="lps")

                qlos = []
                for j in range(jmax):
                    acts = [t for t in tiles if j < extc[t]]
                    assert acts == tiles[-len(acts):], \
                        "active tiles must be a suffix of the group"
                    qlos.append(acts[0] * P - base)

                def emit_score(j):
                    qlo = qlos[j]
                    sc = scp.tile([P, QW], F32, name="sc", tag="sc")
                    nc.tensor.matmul(
                        sc[:, qlo:QW], kro[kv][:, j * P:(j + 1) * P],
                        qkro[h][:, base + qlo:base + QW],
                        start=True, stop=True, skip_group_check=True)
                    for t in tiles:
                        if j >= extc[t]:
                            continue
                        st = blkstate[t][j]
                        if st.startswith("V"):
                            vi = int(st[2:])
                            off = t * P - base
                            nc.vector.tensor_tensor(
                                sc[:, off:off + P], sc[:, off:off + P],
                                varblkT[:, vi * P:(vi + 1) * P],
                                op=AluOpType.add)
                    pt = ptp.tile([P, QW], BF16, name="pt", tag="pt")
                    nc.scalar.activation(
                        pt[:, qlo:QW], sc[:, qlo:QW], AF.Exp,
                        bias=dynT[:, j * HPC + h:j * HPC + h + 1])
                    return pt

                # software-pipeline by two chunks: emit chunk j+1/j+2's
                # score matmuls before chunk j's l/av matmuls so the PE
                # works through the exp latency instead of waiting on it.
                ptq = [emit_score(0)]
                if jmax > 1:
                    ptq.append(emit_score(1))
                for j in range(jmax):
                    pt, qlo = ptq.pop(0), qlos[j]
                    if j + 2 < jmax:
                        ptq.append(emit_score(j + 2))
                    nc.tensor.matmul(
                        lps[:, qlo:QW], onescol_b[:], pt[:, qlo:QW],
                        start=(j == 0), stop=(j == jmax - 1),
                        skip_group_check=True)
                    nc.tensor.matmul(
                        ovp[:, qlo:QW], vnat[kv][:, j * P:(j + 1) * P],
                        pt[:, qlo:QW],
                        start=(j == 0), stop=(j == jmax - 1),
                        skip_group_check=True)
                lsb = lnv.tile([1, QW], F32, name="lsb", tag="lv")
                nc.scalar.copy(lsb[:], lps[:])
                nc.sync.dma_start(l_d[h:h + 1, base:base + QW], lsb[:])
                linv = lnv.tile([1, QW], F32, name="linv", tag="lv")
                nc.vector.reciprocal_approx_fast(linv[:], lps[:])
                lbc = lnb.tile([P, QW], F32, name="lbc", tag="lb")
                nc.gpsimd.partition_broadcast(lbc[:], linv[:])
                at = atn.tile([P, QW], F32R, name="at", tag="at")
                nc.vector.tensor_tensor(at[:], ovp[:], lbc[:],
                                        op=AluOpType.mult)
                at_grp[h] = at
            for ht in range(NT):
                op = ops.tile([P, QW], F32, name="op", tag="op")
                for h in range(HPC):
                    nc.tensor.matmul(op[:], wos[ht][:, h * P:(h + 1) * P],
                                     at_grp[h][:], start=(h == 0),
                                     stop=(h == HPC - 1))
                osb = oub.tile([P, QW], F32, name="osb", tag="ob")
                # alternate the psum drain between scalar and DVE so
                # neither in-order queue delays the next group's exps
                if ht % 2 == 0:
                    nc.scalar.copy(osb[:], op[:])
                else:
                    nc.vector.tensor_copy(osb[:], op[:])
                nc.sync.dma_start(
                    outT_d[ht * P:(ht + 1) * P, base:base + QW], osb[:])
    ctx.close()


def _host_prep(hidden_states, cos, sin, attention_mask, Wq, Wk, Wv, A, Wdt, Wo):
    eye = np.eye(P, dtype=np.float32)
    perm = np.zeros((P, P), dtype=np.float32)
    for j in range(64):
        perm[j + 64, j] = -1.0
        perm[j, j + 64] = 1.0

    def pack_w(wT, nblk):
        # wT [HID, nblk*P] f32 -> [P, nblk*NT*P] bf16:
        # [p, (oi*NT+c)*P+f] = wT[c*P+p, oi*P+f]
        w4 = wT.reshape(NT, P, nblk, P)            # [c, p, oi, f]
        return np.ascontiguousarray(
            w4.transpose(1, 2, 0, 3).reshape(P, nblk * NT * P)
        ).astype(np.float32)

    in_maps = []
    blkstates = []
    for c in range(NCORES):
        b, g = divmod(c, 4)
        heads = list(range(4 * g, 4 * g + 4))
        # x packed: [p, ((sg*NT)+c)*QW+f] = x[b][sg*QW+f, c*P+p]
        xb = np.asarray(hidden_states[b], dtype=np.float32)
        xP = np.ascontiguousarray(
            xb.reshape(NQ, QW, NT, P).transpose(3, 0, 2, 1)
            .reshape(P, NQ * NT * QW))
        wqT = (Wq[4 * g * D:(4 * g + 4) * D] * np.float32(SCALING)).T
        wkT = Wk[2 * g * D:(2 * g + 2) * D].T
        wvT = Wv[2 * g * D:(2 * g + 2) * D].T
        wqP = pack_w(np.ascontiguousarray(wqT), HPC)
        wkP = pack_w(np.ascontiguousarray(wkT), KVPC)
        wvP = pack_w(np.ascontiguousarray(wvT), KVPC)
        wdtvT = np.ascontiguousarray(
            (Wdt[heads].astype(np.float64) @ Wv.astype(np.float64))
            .T.astype(np.float32))                 # [HID, 4]
        wdtvP = np.ascontiguousarray(
            wdtvT.reshape(NT, P, HPC).transpose(1, 0, 2).reshape(P, NT * HPC))
        # woP: [p, (ht*HPC+h)*P+f] = WoT[h*P+p, ht*P+f]
        woT = np.ascontiguousarray(Wo[:, 4 * g * D:(4 * g + 4) * D].T)
        woP = np.ascontiguousarray(
            woT.reshape(HPC, P, NT, P).transpose(1, 2, 0, 3)
            .reshape(P, NT * HPC * P))
        acol = A[heads].astype(np.float32).reshape(HPC, 1)
        cosT = np.ascontiguousarray(cos[b].T)
        sinT = np.ascontiguousarray(sin[b].T)
        m = attention_mask[b, 0]
        mb = np.asarray(m).reshape(NT, P, NT, P)
        blkrows = []
        varlist = []
        varkeys = {}
        for t in range(NT):
            row = []
            for j in range(NT):
                blkv = mb[t, :, j, :]
                if np.all(blkv == 0):
                    row.append("Z")
                elif np.all(blkv <= -1e30):
                    row.append("M")
                else:
                    bT = np.ascontiguousarray(
                        np.maximum(blkv, -BIG).T)  # [key, query]
                    kk = bT.tobytes()
                    if kk not in varkeys:
                        varkeys[kk] = len(varlist)
                        varlist.append(bT)
                    row.append(f"V:{varkeys[kk]}")
            # interior M blocks (before a later non-M block) become varying
            nz = [j for j in range(NT) if row[j] != "M"]
            lim = (max(nz) + 1) if nz else 0
            for j in range(lim):
                if row[j] == "M":
                    bT = np.full((P, P), -BIG, np.float32)
                    kk = bT.tobytes()
                    if kk not in varkeys:
                        varkeys[kk] = len(varlist)
                        varlist.append(bT)
                    row[j] = f"V:{varkeys[kk]}"
            blkrows.append(tuple(row))
        if len(varlist) > 8:
            raise NotImplementedError("too many varying mask blocks")
        varblkT = np.zeros((P, max(len(varlist), 1) * P), dtype=np.float32)
        for vi, blkv in enumerate(varlist):
            varblkT[:, vi * P:(vi + 1) * P] = blkv
        blkstate = tuple(blkrows)
        in_maps.append({
            "xP": xP.astype(BF16NP), "xPf": xP,
            "wqP": wqP.astype(BF16NP),
            "wkP": wkP.astype(BF16NP), "wvP": wvP.astype(BF16NP),
            "wdtvPr": wdtvP, "woP": woP, "cosT": cosT,
            "sinT": sinT, "acol": acol, "eye": eye, "perm": perm,
            "varblkT": varblkT,
        })
        blkstates.append(blkstate)
    if len(set(blkstates)) != 1:
        raise NotImplementedError("mask structure differs across batches")
    return in_maps, blkstates[0]


def _softplus64(x):
    x = x.astype(np.float64)
    return np.log1p(np.exp(-np.abs(x))) + np.maximum(x, 0)


def _repair_rows(out, bad, inputs):
    """Recompute rows flagged bad [B, S] with faithful numpy reference math."""
    if not bad.any():
        return out
    hs = inputs["hidden_states"]; cos = inputs["cos"]; sin = inputs["sin"]
    am = inputs["attention_mask"]; Wq = inputs["Wq"]; Wk = inputs["Wk"]
    Wv = inputs["Wv"]; A = inputs["A"]; Wdt = inputs["Wdt"]; Wo = inputs["Wo"]

    def rope(x, c, s):
        x1, x2 = x[..., :D // 2], x[..., D // 2:]
        return x * c + np.concatenate([-x2, x1], axis=-1) * s

    for b in range(B):
        rows = np.where(bad[b])[0]
        if len(rows) == 0:
            continue
        x = hs[b].astype(np.float32)
        k = (x @ Wk.T).reshape(S, KV, D)
        v = (x @ Wv.T).reshape(S, KV, D)
        k = rope(k, cos[b][:, None, :], sin[b][:, None, :])
        v_flat = v.reshape(S, KV * D)
        dt = v_flat @ Wdt.T
        dyn = np.exp(A[None, :] * _softplus64(dt)).astype(np.float32).T
        kth = np.sort(dyn, axis=-1)[:, NUM_DYN - 1:NUM_DYN]
        dmask = np.where(dyn < kth, MIN, dyn).astype(np.float32)
        for s_i in rows:
            q_row = (x[s_i] @ Wq.T).reshape(H, D)
            q_row = rope(q_row, cos[b][s_i][None, :], sin[b][s_i][None, :])
            attn_row = np.zeros((H, D), dtype=np.float32)
            for h in range(H):
                kvh = h // GROUPS
                sc = ((q_row[h] @ k[:, kvh].T) * np.float32(SCALING)
                      + (dmask[h] + am[b, 0, s_i])).astype(np.float32)
                w = np.exp(sc - sc.max())
                w = (w / w.sum()).astype(np.float32)
                attn_row[h] = w @ v[:, kvh]
            out[b, s_i] = attn_row.reshape(H * D) @ Wo.T
    return out


def kernel(**inputs):
    inputs = {k: np.asarray(v) for k, v in inputs.items()}
    in_maps, blkstate = _host_prep(**inputs)
    nc = _build_program(blkstate)
    res = run_bass_kernel_spmd(nc, in_maps, list(range(NCORES)))
    out = np.zeros((B, S, HID), dtype=np.float32)
    bad = np.zeros((B, S), dtype=bool)
    for c in range(NCORES):
        b = c // 4
        out[b] += res.results[c]["outT"].T
        bad[b] |= (res.results[c]["l_out"] == 0).any(axis=0)
    bad |= ~np.isfinite(out).all(axis=2)
    out = _repair_rows(out, bad, inputs)
    return out
